# revision 3
# baseline (speedup 1.0000x reference)
"""Trainium2 Bass kernel for batched cross-attention (CoupletsAttentionModel).

Reference computation (per batch element b):
    S = dec @ enc^T          [S_dec, S_enc]
    P = softmax(S, axis=-1)
    O = P @ enc              [S_dec, D]

Sharding: data-parallel over batch — B=8 batch elements, one per NeuronCore.
Each core runs an identical (SPMD) program on its own batch slice; no
collectives, host stacks the 8 per-core outputs.

Per-core algorithm (S_enc=S_dec=2048, D=512, fp32 in/out):
  - Load enc/dec fp32, cast to fp16, build enc^T / dec^T via PE transposes
    (fp16 matmuls run 4x faster than fp32 and fit accuracy budget:
    scores |S| <~ 120, fp16 dot error ~7e-3 rms -> output rel err ~2e-3).
  - Per 128-row q-tile: S row-block [128, 2048] fp32 in 4 PSUM banks
    (16 fp16 matmuls), row-max on VectorE, exp(S - max) + row-sum fused on
    ScalarE (accum_out) writing fp16 P, P^T via PE transpose, P^T @ V
    accumulated over 16 k-tiles in PSUM, final 1/sum scale on VectorE,
    DMA out.
"""

import contextlib
import ctypes
import os
import sys
import types

import numpy as np

import concourse.bass as bass
import concourse.tile as tile
from concourse import bacc, mybir
from concourse import bass_utils
from concourse.masks import make_identity

F32 = mybir.dt.float32
F16 = mybir.dt.float16
AX = mybir.AxisListType
AFT = mybir.ActivationFunctionType

N_CORES = 8
PART = 128


def attention_tile_kernel(tc, out_ap, dec_ap, enc_ap, seq, d):
    nc = tc.nc
    P = PART
    KC = 512  # k-chunk (matmul free dim / one PSUM bank of fp32)
    n_qt = seq // P  # q tiles
    n_kt = seq // P  # k tiles
    n_kc = seq // KC  # k chunks
    n_dt = d // P  # d tiles (contraction tiles for S)

    stack = contextlib.ExitStack()
    pool = lambda **kw: stack.enter_context(tc.tile_pool(**kw))

    singles = pool(name="singles", bufs=1)
    big = pool(name="big", bufs=1)
    stage = pool(name="stage", bufs=3)
    stage16 = pool(name="stage16", bufs=3)
    pt_ps = pool(name="pt_ps", bufs=2, space="PSUM")
    s_ps = pool(name="s_ps", bufs=4, space="PSUM")
    o_ps = pool(name="o_ps", bufs=2, space="PSUM")
    psmall = pool(name="psmall", bufs=2)
    ptT_pool = pool(name="ptT", bufs=3)
    stats = pool(name="stats", bufs=4)
    osb = pool(name="osb", bufs=2)

    with stack:
        ident = singles.tile([P, P], F16)
        make_identity(nc, ident[:])

        v_sb = big.tile([P, n_kt, d], F16)  # enc natural (V), f16
        kT_sb = big.tile([P, n_dt, seq], F16)  # enc^T  [d_part, dt, k]
        qT_sb = big.tile([P, n_dt, seq], F16)  # dec^T  [d_part, dt, q]

        # ---- load enc: cast to f16 (V) and transpose into kT_sb ----
        for st in range(n_kt):
            e32 = stage.tile([P, d], F32, tag="ld32")
            nc.sync.dma_start(out=e32[:], in_=enc_ap[st * P : (st + 1) * P, :])
            nc.vector.tensor_copy(v_sb[:, st, :], e32[:])
            for dt_ in range(n_dt):
                ps = pt_ps.tile([P, P], F16, tag="tps")
                nc.tensor.transpose(
                    ps[:], v_sb[:, st, dt_ * P : (dt_ + 1) * P], ident[:]
                )
                nc.vector.tensor_copy(kT_sb[:, dt_, st * P : (st + 1) * P], ps[:])

        # ---- load dec: cast + transpose into qT_sb ----
        for st in range(n_qt):
            d32 = stage.tile([P, d], F32, tag="ld32")
            nc.sync.dma_start(out=d32[:], in_=dec_ap[st * P : (st + 1) * P, :])
            d16 = stage16.tile([P, d], F16)
            nc.vector.tensor_copy(d16[:], d32[:])
            for dt_ in range(n_dt):
                ps = pt_ps.tile([P, P], F16, tag="tps")
                nc.tensor.transpose(ps[:], d16[:, dt_ * P : (dt_ + 1) * P], ident[:])
                nc.vector.tensor_copy(qT_sb[:, dt_, st * P : (st + 1) * P], ps[:])

        # ---- main loop over q tiles ----
        for qt in range(n_qt):
            q_lo = qt * P
            # S row-block: n_kc chunks of [128, KC] fp32 in PSUM
            s_chunks = []
            for kc in range(n_kc):
                s_ch = s_ps.tile([P, KC], F32, tag="s_ch")
                for dt_ in range(n_dt):
                    nc.tensor.matmul(
                        s_ch[:],
                        qT_sb[:, dt_, q_lo : q_lo + P],
                        kT_sb[:, dt_, kc * KC : (kc + 1) * KC],
                        start=(dt_ == 0),
                        stop=(dt_ == n_dt - 1),
                    )
                s_chunks.append(s_ch)

            # row max (negated, for exp bias)
            mx = stats.tile([P, n_kc], F32, tag="mx")
            for kc in range(n_kc):
                nc.vector.reduce_max(mx[:, kc : kc + 1], s_chunks[kc][:], axis=AX.X)
            negm = stats.tile([P, 1], F32, tag="negm")
            nc.vector.tensor_reduce(
                negm[:], mx[:], axis=AX.X, op=mybir.AluOpType.max, negate=True
            )

            # P = exp(S - max) as f16, with fused row-sum chunks
            p_sb = psmall.tile([P, seq], F16)
            sums = stats.tile([P, n_kc], F32, tag="sums")
            for kc in range(n_kc):
                nc.scalar.activation(
                    p_sb[:, kc * KC : (kc + 1) * KC],
                    s_chunks[kc][:],
                    AFT.Exp,
                    bias=negm[:],
                    scale=1.0,
                    accum_out=sums[:, kc : kc + 1],
                )
            sm = stats.tile([P, 1], F32, tag="sm")
            nc.vector.reduce_sum(sm[:], sums[:], axis=AX.X)
            rinv = stats.tile([P, 1], F32, tag="rinv")
            nc.vector.reciprocal(rinv[:], sm[:])

            # O = P @ V, accumulating over k tiles; P^T via PE transpose
            o_ch = o_ps.tile([P, d], F32)
            for kt in range(n_kt):
                ps = pt_ps.tile([P, P], F16, tag="tps")
                nc.tensor.transpose(ps[:], p_sb[:, kt * P : (kt + 1) * P], ident[:])
                pT = ptT_pool.tile([P, P], F16)
                nc.vector.tensor_copy(pT[:], ps[:])
                nc.tensor.matmul(
                    o_ch[:],
                    pT[:],
                    v_sb[:, kt, :],
                    start=(kt == 0),
                    stop=(kt == n_kt - 1),
                )

            o_sb = osb.tile([P, d], F32)
            nc.vector.tensor_scalar_mul(o_sb[:], o_ch[:], rinv[:])
            nc.sync.dma_start(out=out_ap[q_lo : q_lo + P, :], in_=o_sb[:])


def build(seq=2048, d=512, n_cores=N_CORES):
    nc = bacc.Bacc(
        "TRN2", target_bir_lowering=False, debug=False, num_devices=n_cores
    )
    dec = nc.dram_tensor("dec", [seq, d], F32, kind="ExternalInput").ap()
    enc = nc.dram_tensor("enc", [seq, d], F32, kind="ExternalInput").ap()
    out = nc.dram_tensor("out", [seq, d], F32, kind="ExternalOutput").ap()
    with tile.TileContext(nc) as tc:
        attention_tile_kernel(tc, out, dec, enc, seq, d)
    nc.compile()
    return nc


# ---------------------------------------------------------------------------
# Optional NTFF profiling support (used by our own test harness; inert unless
# BASSKERNEL_TRACE=1). The agent image lacks `antenv.axon_hooks`, so recreate
# it in sys.modules with a ctypes hook against libaxon_pjrt.so.
# ---------------------------------------------------------------------------
LAST_EXEC_TIME_NS = None


def _install_profile_hook():
    so_path = "/opt/axon/libaxon_pjrt.so"
    if "antenv.axon_hooks" in sys.modules or not os.path.exists(so_path):
        return
    lib = ctypes.CDLL(so_path)
    if not hasattr(lib, "axon_start_nrt_profile"):
        return
    lib.axon_start_nrt_profile.argtypes = [
        ctypes.POINTER(ctypes.c_int64),
        ctypes.c_size_t,
    ]
    lib.axon_start_nrt_profile.restype = ctypes.c_int64
    lib.axon_stop_nrt_profile.argtypes = [ctypes.c_char_p]
    lib.axon_stop_nrt_profile.restype = ctypes.c_int64

    @contextlib.contextmanager
    def _hook(output_dir, device_ids):
        import jax

        jax.devices()
        if device_ids:
            ids = (ctypes.c_int64 * len(device_ids))(*device_ids)
            rc = lib.axon_start_nrt_profile(ids, len(device_ids))
        else:
            rc = lib.axon_start_nrt_profile(None, 0)
        if rc != 0:
            raise RuntimeError(f"axon_start_nrt_profile rc={rc}")
        try:
            yield
        finally:
            n = lib.axon_stop_nrt_profile(str(output_dir).encode())
            print(f"ntff profile: {n} file(s) written to {output_dir}")

    mod = types.ModuleType("antenv.axon_hooks")
    _state = {"hook": _hook}
    mod.set_axon_ntff_profile_hook = lambda h: _state.__setitem__("hook", h)
    mod.get_axon_ntff_profile_hook = lambda: _state["hook"]
    sys.modules["antenv.axon_hooks"] = mod
    bass_utils.upload_artifacts = lambda tmpdir: tmpdir


_NC_CACHE = {}


def kernel(enc_outputs: np.ndarray, dec_outputs: np.ndarray) -> np.ndarray:
    B, seq, d = dec_outputs.shape
    assert enc_outputs.shape == (B, seq, d) and B == N_CORES

    trace = os.environ.get("BASSKERNEL_TRACE", "0") == "1"
    if trace:
        _install_profile_hook()

    key = (seq, d)
    if key not in _NC_CACHE:
        _NC_CACHE[key] = build(seq, d)
    nc = _NC_CACHE[key]

    in_maps = [
        {
            "dec": np.ascontiguousarray(dec_outputs[b], dtype=np.float32),
            "enc": np.ascontiguousarray(enc_outputs[b], dtype=np.float32),
        }
        for b in range(B)
    ]
    res = bass_utils.run_bass_kernel_spmd(
        nc,
        in_maps,
        core_ids=list(range(N_CORES)),
        trace=trace,
        tmpdir=os.environ.get("BASSKERNEL_TRACE_DIR") if trace else None,
    )
    global LAST_EXEC_TIME_NS
    LAST_EXEC_TIME_NS = res.exec_time_ns
    out = np.stack([res.results[b]["out"] for b in range(B)], axis=0)
    return out.astype(np.float32)
